# revision 18
# baseline (speedup 1.0000x reference)
"""Grouped-experts SwiGLU FFN (MoE) on 8 Trainium2 NeuronCores.

Expert-parallel: core e owns expert e's weights and its contiguous token
slice.  Tokens are already sorted by expert (contiguous ranges from
cumsum(num_tokens_per_expert)), so the "all-to-all dispatch" is plain host
slicing.  Each core runs a two-stage SwiGLU:

  stage 1:  HT[h, t] = silu(W1 x)[h, t] * (W3 x)[h, t]     (K = DIM)
  stage 2:  OUT.T[d, t] = (W2 @ H)[d, t]                   (K = HIDDEN)

Matmuls run in bf16 (1 col/cycle on the PE array, fp32 PSUM accumulate);
~4e-3 relative error vs the fp32 reference.

Schedule notes (from NTFF traces):
 - The MM stream runs at the 216 ns/MM roofline (N=512 @ 2.4 GHz + NX
   overhead) once data is resident; all wins are at the edges.
 - Stage 1 iterates token-chunk-outer / hidden-tile-inner with w1/w3 fully
   resident in SBUF (11.5 MB), so early HBM demand is ~220 GB/s instead of
   the >400 GB/s ramp the chunk-inner order needs (which stalled ~5 us and
   re-throttled the PE clock).
 - DMA submissions cost ~650 ns each on a sequencer; they are split across
   the two HWDGE queues (Sync: weights, Scalar: x + outputs) and the first
   blocks are tiny so the first real matmul issues ~1.4 us after the fixed
   ~7 us NEFF preamble.
 - A few dep-free dummy matmuls bridge the remaining DMA wait and start the
   HAM clock warm-up.
 - Output is written as bf16 (halves out traffic; adds ~2e-4 rel error) and
   the last chunk is split in two flushed on both queues to shorten the tail.
"""

import numpy as np
import ml_dtypes

import concourse.bass as bass
from concourse import bacc
import concourse.mybir as mybir
from concourse.tile import TileContext
from concourse.bass_utils import run_bass_kernel_spmd

N_TOKENS = 16384
DIM = 2048
HIDDEN = 1408
N_EXPERTS = 8
N_CORES = 8

P = 128
T = 2048                 # token capacity per core per pass
N_DN = DIM // P          # 16 contraction blocks in stage 1
N_HT = HIDDEN // P       # 11 h tiles
N_DT = DIM // P          # 16 output-row tiles in stage 2
TSUB = 512               # moving-operand width per matmul (1 PSUM bank)
NTS = T // TSUB          # 4 token chunks

F32 = mybir.dt.float32
BF16 = mybir.dt.bfloat16
SILU = mybir.ActivationFunctionType.Silu
BF = ml_dtypes.bfloat16


def _build_program() -> bass.Bass:
    nc = bacc.Bacc(enable_partition_id=False)
    xtp = nc.declare_dram_parameter(
        "xtp", [P, NTS, N_DN, TSUB], BF16, isOutput=False)
    w1p = nc.declare_dram_parameter("w1p", [P, N_HT, N_DN, P], BF16, isOutput=False)
    w3p = nc.declare_dram_parameter("w3p", [P, N_HT, N_DN, P], BF16, isOutput=False)
    w2p = nc.declare_dram_parameter("w2p", [P, N_DT, N_HT, P], BF16, isOutput=False)
    outt = nc.declare_dram_parameter("outt", [DIM, T], BF16, isOutput=True)

    with TileContext(nc) as tc:
        with (
            tc.tile_pool(name="xt", bufs=2) as xt_pool,
            tc.tile_pool(name="w1", bufs=1) as w1_pool,
            tc.tile_pool(name="w3", bufs=1) as w3_pool,
            tc.tile_pool(name="ht", bufs=1) as ht_pool,
            tc.tile_pool(name="w2", bufs=3) as w2_pool,
            tc.tile_pool(name="tmp", bufs=3) as tmp_pool,
            tc.tile_pool(name="ob", bufs=4) as ob_pool,
            tc.tile_pool(name="ps1", bufs=2, space="PSUM") as ps1_pool,
            tc.tile_pool(name="ps2", bufs=2, space="PSUM") as ps2_pool,
            tc.tile_pool(name="pso", bufs=2, space="PSUM") as pso_pool,
            tc.tile_pool(name="dmy", bufs=1) as dmy_pool,
            tc.tile_pool(name="psd", bufs=2, space="PSUM") as psd_pool,
        ):
            # PE pre-warm: dep-free dummy matmuls bridge the initial DMA wait
            # and release the HAM clock throttle (~3.4 us of busy) before
            # real work
            dmy = dmy_pool.tile([P, TSUB], BF16)
            nc.vector.memset(dmy[:], 0.0)

            def dummy_mm():
                psd = psd_pool.tile([P, TSUB], F32)
                nc.tensor.matmul(psd[:], lhsT=dmy[:, 0:P], rhs=dmy[:],
                                 skip_group_check=True)

            for _ in range(4):
                dummy_mm()

            # resident stage-1 weights: [P, ih, n, P]
            w1all = w1_pool.tile([P, N_HT, N_DN, P], BF16)
            w3all = w3_pool.tile([P, N_HT, N_DN, P], BF16)
            ht = ht_pool.tile([P, N_HT, T], BF16)

            # Early DMA: the cold PE consumes w1[ih0]+xc0 (2.5 MB) in ~7 us —
            # right at the HBM roofline — so submissions are staggered by
            # need-time across both HWDGE queues (Sync: weights, Scalar: x)
            # and everything not needed in the first two groups (w3, ih>=1)
            # is queued behind them.
            xcs: list = [None] * NTS
            xcs[0] = xt_pool.tile([P, N_DN, TSUB], BF16, tag="xt", name="xc0")
            # quarter-granularity first loads, staggered by need-time
            for sl in (slice(0, 2), slice(2, 6), slice(6, 11), slice(11, 16)):
                nc.sync.dma_start(out=w1all[:, 0, sl, :], in_=w1p[:, 0, sl, :])
                nc.scalar.dma_start(out=xcs[0][:, sl, :], in_=xtp[:, 0, sl, :])
            nc.sync.dma_start(out=w3all[:, 0, 0:4, :], in_=w3p[:, 0, 0:4, :])
            nc.sync.dma_start(out=w3all[:, 0, 4:16, :], in_=w3p[:, 0, 4:16, :])
            for ih in range(1, N_HT):
                nc.sync.dma_start(out=w1all[:, ih, :, :], in_=w1p[:, ih, :, :])
                nc.sync.dma_start(out=w3all[:, ih, :, :], in_=w3p[:, ih, :, :])

            # stage 1: HT[h, t] = silu(x @ w1.T).T * (x @ w3.T).T
            for its in range(NTS):
                xt_c = xcs[its]
                ts0 = its * TSUB
                for ih in range(N_HT):
                    # prefetch the next token chunk mid-iteration so its HBM
                    # traffic doesn't compete with the weight stream early on
                    if ih == 5 and its + 1 < NTS:
                        xcs[its + 1] = xt_pool.tile(
                            [P, N_DN, TSUB], BF16, tag="xt", name=f"xc{its + 1}")
                        nc.scalar.dma_start(
                            out=xcs[its + 1][:], in_=xtp[:, its + 1, :, :])
                    # the very first groups are paced by HBM arrival (~2 MM
                    # slots per block); interleaved dummy matmuls keep the PE
                    # busy through the arrival gaps so the HAM clock throttle
                    # never re-engages
                    pace = its == 0 and ih == 0
                    ps1 = ps1_pool.tile([P, TSUB], F32)
                    ps2 = ps2_pool.tile([P, TSUB], F32)
                    for n in range(N_DN):
                        nc.tensor.matmul(
                            ps1[:],
                            lhsT=w1all[:, ih, n, :],
                            rhs=xt_c[:, n, :],
                            start=(n == 0),
                            stop=(n == N_DN - 1),
                            skip_group_check=pace,
                        )
                        if pace and n < 14:
                            dummy_mm()
                    if pace:
                        for _ in range(5):
                            dummy_mm()
                    for n in range(N_DN):
                        nc.tensor.matmul(
                            ps2[:],
                            lhsT=w3all[:, ih, n, :],
                            rhs=xt_c[:, n, :],
                            start=(n == 0),
                            stop=(n == N_DN - 1),
                        )
                    tmp = tmp_pool.tile([P, TSUB], F32)
                    nc.scalar.activation(tmp[:], ps1[:], SILU)
                    nc.vector.tensor_mul(
                        ht[:, ih, ts0:ts0 + TSUB], tmp[:], ps2[:]
                    )

            # stage 2: OUT.T[d, t] = sum_h W2T[h, d] * HT[h, t]
            COPY = mybir.ActivationFunctionType.Copy
            H = TSUB // 2
            for idt in range(N_DT):
                w2b = w2_pool.tile([P, N_HT, P], BF16)
                nc.sync.dma_start(out=w2b[:], in_=w2p[:, idt, :, :])
                for its in range(NTS):
                    ts0 = its * TSUB
                    last = (idt == N_DT - 1 and its == NTS - 1)
                    if not last:
                        pso = pso_pool.tile([P, TSUB], F32, tag="pso")
                        for hn in range(N_HT):
                            nc.tensor.matmul(
                                pso[:],
                                lhsT=w2b[:, hn, :],
                                rhs=ht[:, hn, ts0:ts0 + TSUB],
                                start=(hn == 0),
                                stop=(hn == N_HT - 1),
                            )
                        ob = ob_pool.tile([P, TSUB], BF16)
                        nc.vector.tensor_copy(ob[:], pso[:])
                        nc.scalar.dma_start(
                            out=outt[idt * P:(idt + 1) * P, ts0:ts0 + TSUB],
                            in_=ob[:],
                        )
                    else:
                        # final group: two half-width PSUM groups so the first
                        # half's flush overlaps the second half's matmuls, and
                        # the last flush is small, split across engines/queues
                        for h in range(2):
                            pso = pso_pool.tile([P, H], F32, tag="pso",
                                                name=f"psoh{h}")
                            for hn in range(N_HT):
                                nc.tensor.matmul(
                                    pso[:],
                                    lhsT=w2b[:, hn, :],
                                    rhs=ht[:, hn,
                                           ts0 + h * H:ts0 + (h + 1) * H],
                                    start=(hn == 0),
                                    stop=(hn == N_HT - 1),
                                )
                            if h == 0:
                                ob = ob_pool.tile([P, H], BF16, name="ha")
                                nc.vector.tensor_copy(ob[:], pso[:])
                                nc.scalar.dma_start(
                                    out=outt[idt * P:(idt + 1) * P,
                                             ts0:ts0 + H],
                                    in_=ob[:],
                                )
                            else:
                                hb = ob_pool.tile([P, H], BF16, name="hb")
                                nc.scalar.activation(hb[:], pso[:], COPY)
                                nc.sync.dma_start(
                                    out=outt[idt * P:(idt + 1) * P,
                                             ts0 + H:ts0 + TSUB],
                                    in_=hb[:],
                                )
    nc.compile()
    return nc


_CACHE: dict = {}


def _get_nc() -> bass.Bass:
    if "nc" not in _CACHE:
        _CACHE["nc"] = _build_program()
    return _CACHE["nc"]


def _pack_weights(w1, w2, w3):
    maps = []
    for e in range(N_EXPERTS):
        maps.append({
            "w1p": np.ascontiguousarray(
                w1[e].reshape(N_HT, P, N_DN, P).transpose(3, 0, 2, 1).astype(BF)),
            "w3p": np.ascontiguousarray(
                w3[e].reshape(N_HT, P, N_DN, P).transpose(3, 0, 2, 1).astype(BF)),
            "w2p": np.ascontiguousarray(
                w2[e].reshape(N_DT, P, N_HT, P).transpose(3, 0, 2, 1).astype(BF)),
        })
    return maps


def kernel(x, w1, w2, w3, num_tokens_per_expert, _trace=False):
    x = np.ascontiguousarray(np.asarray(x, dtype=np.float32))
    w1 = np.ascontiguousarray(np.asarray(w1, dtype=np.float32))
    w2 = np.ascontiguousarray(np.asarray(w2, dtype=np.float32))
    w3 = np.ascontiguousarray(np.asarray(w3, dtype=np.float32))
    counts = np.asarray(num_tokens_per_expert, dtype=np.int64)

    cs = np.cumsum(counts)
    starts = np.minimum(np.concatenate([[0], cs[:-1]]), N_TOKENS)
    ends = np.minimum(cs, N_TOKENS)
    lens = np.maximum(ends - starts, 0)

    wmaps = _pack_weights(w1, w2, w3)
    out = np.zeros((N_TOKENS, DIM), np.float32)
    trace_info = []

    n_passes = max(1, int(np.max(np.ceil(lens / T))))
    for k in range(n_passes):
        in_maps = []
        for e in range(N_EXPERTS):
            s = int(starts[e]) + k * T
            xe = np.zeros((T, DIM), np.float32)
            avail = x[s:s + T]
            if avail.shape[0]:
                xe[:avail.shape[0]] = avail
            # [P, n_chunks, N_DN, TSUB]: xtp[p, c, n, t] = x[c*TSUB+t, n*128+p]
            xtp = np.ascontiguousarray(
                xe.T.reshape(N_DN, P, NTS, TSUB)
                .transpose(1, 2, 0, 3).astype(BF))
            in_maps.append({"xtp": xtp, **wmaps[e]})
        res = run_bass_kernel_spmd(
            _get_nc(), in_maps, list(range(N_CORES)), trace=_trace
        )
        if _trace:
            trace_info.append(res)
        for e in range(N_EXPERTS):
            s = int(starts[e]) + k * T
            cnt = min(int(ends[e]) - s, T)
            if cnt > 0:
                out[s:s + cnt] = res.results[e]["outt"].T[:cnt].astype(np.float32)

    if _trace:
        return out, trace_info
    return out
